# revision 15
# baseline (speedup 1.0000x reference)
"""Cross-attention kernel for Trainium2, SPMD over 8 NeuronCores.

Problem (hardcoded): B=32, N=2560 queries, Dq=512, Dc=1024, 8 heads x 64 dim,
context = 77 text + 16 image tokens, two attentions (text keys via W_k/W_v,
image keys via W_k_ip/W_v_ip) summed, then W_out projection + bias.

Sharding: data-parallel over batch, 4 batches per core, no collectives.
x and context are transposed host-side during sharding so every DMA lands in
the feature-on-partitions layout the matmuls need.

All matmul operands are fp16 (1 cycle/row on PE at ANY moving size, vs
fp32r's 4x penalty under 256; fp32 PSUM accumulate keeps precision ~1e-3).

Per-core kernel, per 512-query chunk (software-pipelined over chunks):
  P: q^T = W_q^T @ x^T                            (PE; Act evacuates)
  A: s^T[93 keys, 512 q] = k^T_h^T @ q^T_h; E = exp(s^T) -> fp16 (ScalarE)
  B: r[2, 512] = ind^T @ E  (txt/img key sums, PE); r_inv = 1/r (VectorE,
     all 8 heads into one [2,8,512] tile); r_inv is then replicated across
     key partitions by TWO partition_broadcast DMAs (txt rows 0:77 get
     r_inv[0], img rows 77:93 get r_inv[1]) -- this replaces the old
     broadcast matmul, saving 8x512 PE rows per chunk.
  C: E *= rb (VectorE, fp16)
  D: O^T = V^T @ E  (single matmul over all 93 keys sums the text and
     image attention outputs); Pool engine evacuates to attn^T fp16
  F: out = attn^T^T @ W_out; Pool adds bias, fp16 -> DMA out

Emission interleaves A/P and B/D on the PE stream so the in-order engine
never stalls on Act/DVE evacuation pacing.

PSUM budget (8 banks): big(q/final)=2, scores/attn=4, sums=2.
"""

import sys

if "/opt/trn_rl_repo" not in sys.path:
    sys.path.insert(0, "/opt/trn_rl_repo")

import numpy as np

from concourse import bacc
import concourse.mybir as mybir
from concourse.tile import TileContext
from concourse.bass_utils import run_bass_kernel_spmd

F32 = mybir.dt.float32
F16 = mybir.dt.float16
EXP = mybir.ActivationFunctionType.Exp

P = 128
NCORES = 8
B = 32
BPC = B // NCORES  # batches per core
N = 2560
DQ = 512
DC = 1024
H = 8
D = 64
INNER = H * D  # 512
TT = 77  # text tokens
TI = 16  # image tokens
T = TT + TI  # 93
# padded key layout: text keys at rows 0:77, zero pad 77:96, img at 96:112
# (engine partition windows: start 0 spans freely; start 96 spans <=32)
IMS = 96  # img key start row
TX = IMS + TI  # 112 total key rows
CH = 512  # query chunk
NCH = N // CH  # 5
SCALE = D ** (-0.5)

_CACHED = None


def _build(cfg=None):
    cfg = cfg or {}
    import contextlib
    nc = bacc.Bacc("TRN2", target_bir_lowering=False, debug=False, num_devices=NCORES)

    xt_d = nc.dram_tensor("xT", [BPC, DQ, N], F16, kind="ExternalInput").ap()
    ctxt_d = nc.dram_tensor("ctxT", [BPC, DC, T], F16, kind="ExternalInput").ap()
    wq_d = nc.dram_tensor("W_q", [DQ, INNER], F16, kind="ExternalInput").ap()
    wk_d = nc.dram_tensor("W_k", [DC, INNER], F16, kind="ExternalInput").ap()
    wv_d = nc.dram_tensor("W_v", [DC, INNER], F16, kind="ExternalInput").ap()
    wkip_d = nc.dram_tensor("W_k_ip", [DC, INNER], F16, kind="ExternalInput").ap()
    wvip_d = nc.dram_tensor("W_v_ip", [DC, INNER], F16, kind="ExternalInput").ap()
    wout_d = nc.dram_tensor("W_out", [INNER, DQ], F16, kind="ExternalInput").ap()
    ind_d = nc.dram_tensor("ind", [TX, 2], F16, kind="ExternalInput").ap()
    ind2_d = nc.dram_tensor("ind2", [2, TX], F16, kind="ExternalInput").ap()
    ctxi_d = nc.dram_tensor("ctxI", [DC, BPC * TI], F16, kind="ExternalInput").ap()
    bb_d = nc.dram_tensor("b_bcast", [P, DQ], F32, kind="ExternalInput").ap()
    out_d = nc.dram_tensor("out", [BPC, N, DQ], F16, kind="ExternalOutput").ap()

    with TileContext(nc) as tc:
        with (
            tc.tile_pool(name="persist", bufs=1) as pp,
            tc.tile_pool(name="ps_big", bufs=2, space="PSUM") as ps_big,
            tc.tile_pool(name="ps_ss", bufs=3, space="PSUM") as ps_ss,
            tc.tile_pool(name="ps_b", bufs=2, space="PSUM") as ps_b,
            tc.tile_pool(name="ps_r", bufs=1, space="PSUM") as ps_r,
        ):
            ind_t = pp.tile([TX, 2], F16, tag="ind")
            ind2_t = pp.tile([66, TX], F16, tag="ind2")
            bb_t = pp.tile([P, DQ], F32, tag="bb")

            wq_all = pp.tile([P, 4, INNER], F16, tag="wq_all")
            wout_all = pp.tile([P, 4, DQ], F16, tag="wout_all")

            # K^T[b][m] : [128 inner-dims, 93 keys] (text keys 0:77 from W_k,
            # image keys 77:93 from W_k_ip), pre-scaled by 1/sqrt(d).
            # V[b] : [93 keys, 512 inner] (text rows via W_v, image via W_v_ip)
            kT = [
                [
                    pp.tile([P, TX], F16, tag=f"kT{b}_{m}", name=f"kT{b}_{m}")
                    for m in range(4)
                ]
                for b in range(BPC)
            ]
            V = [pp.tile([TX, INNER], F16, tag=f"v{b}", name=f"v{b}") for b in range(BPC)]

            # ---- pools for the main loop (opened early so chunk (0,0)
            # projection work can interleave with phase 0) ----
            wstack = contextlib.ExitStack()
            wp = wstack.enter_context(tc.tile_pool(name="work", bufs=2))
            xsp = wstack.enter_context(tc.tile_pool(name="xsp", bufs=4))
            osp = wstack.enter_context(tc.tile_pool(name="osp", bufs=6))
            ep = wstack.enter_context(tc.tile_pool(name="ework", bufs=18))
            rp = wstack.enter_context(tc.tile_pool(name="rwork", bufs=2))

            xtiles = {}

            def fetch_x(b, c):
                # x^T for a chunk straight from DRAM: [128, kt, 512] fp16
                xT = xsp.tile([P, 4, CH], F16, tag="xT", name=f"xT{b}_{c}")
                nc.sync.dma_start(
                    xT[:],
                    xt_d[b].rearrange("(k p) t -> p k t", p=P)[
                        :, :, c * CH : (c + 1) * CH
                    ],
                )
                xtiles[(b, c)] = xT

            def emit_p(b, c):
                xT = xtiles.pop((b, c))
                # q^T chunk: [128, m, 512] fp16
                qT = wp.tile([P, 4, CH], F16, tag="qT", name=f"qT{b}_{c}")
                for m in range(4):
                    psq = ps_big.tile([P, CH], F32, tag="big", name=f"psq{b}_{c}_{m}")
                    for kt in range(4):
                        nc.tensor.matmul(
                            psq[:],
                            lhsT=wq_all[:, kt, m * P : (m + 1) * P],
                            rhs=xT[:, kt, :],
                            start=(kt == 0),
                            stop=(kt == 3),
                        )
                    nc.scalar.copy(qT[:, m, :], psq[:])
                return (b, c, qT)

            # ---- phase 0: context projections ----
            with tc.tile_pool(name="ph0", bufs=1) as p0:
                wk_all = p0.tile([P, 8, INNER], F16, tag="wk", name="wk_all")
                nc.sync.dma_start(wk_all[:], wk_d.rearrange("(k p) n -> p k n", p=P))
                # all 4 batches' context: [128, kt, b, 93]; plus a host-
                # prebatched image-token view [128, kt, b*16+i] whose single
                # contiguous free dim feeds the batched V_ip projection.
                ctxT = p0.tile([P, 8, BPC, T], F16, tag="ctxT", name="ctxT")
                for b in range(BPC):
                    nc.sync.dma_start(
                        ctxT[:, :, b, :],
                        ctxt_d[b].rearrange("(k p) t -> p k t", p=P),
                    )
                ctxI = p0.tile([P, 8, BPC * TI], F16, tag="ctxI", name="ctxI")
                nc.sync.dma_start(
                    ctxI[:], ctxi_d.rearrange("(k p) n -> p k n", p=P)
                )
                wv_all = p0.tile([P, 8, INNER], F16, tag="wv", name="wv_all")
                nc.sync.dma_start(wv_all[:], wv_d.rearrange("(k p) n -> p k n", p=P))
                nc.sync.dma_start(wq_all[:], wq_d.rearrange("(k p) n -> p k n", p=P))
                fetch_x(0, 0)
                wkip_all = p0.tile([P, 8, INNER], F16, tag="wkip", name="wkip_all")
                nc.sync.dma_start(
                    wkip_all[:], wkip_d.rearrange("(k p) n -> p k n", p=P)
                )
                wvip_all = p0.tile([P, 8, INNER], F16, tag="wvip", name="wvip_all")
                nc.sync.dma_start(
                    wvip_all[:], wvip_d.rearrange("(k p) n -> p k n", p=P)
                )
                fetch_x(0, 1)
                nc.sync.dma_start(
                    wout_all[:], wout_d.rearrange("(k p) n -> p k n", p=P)
                )
                nc.sync.dma_start(ind_t[:], ind_d)
                nc.sync.dma_start(ind2_t[0:2, :], ind2_d)
                nc.sync.dma_start(ind2_t[64:66, :], ind2_d)
                nc.sync.dma_start(bb_t[:], bb_d)

                # zero kT pad cols and V img+pad rows (img values land in
                # V[0:16] afterwards; the DMA-write ordering keeps them)
                for b in range(BPC):
                    for m in range(4):
                        nc.vector.memset(kT[b][m][:, TT:IMS], 0.0)
                    nc.gpsimd.memset(V[b][64:IMS, :], 0.0)

                # text keys: kT[:, 0:TT] (project 78 keys, junk col 77 unused)
                for b in range(BPC):
                    for m in range(4):
                        pst = ps_ss.tile([P, CH], F32, tag="pss")
                        for kt in range(8):
                            nc.tensor.matmul(
                                pst[:, : TT + 1],
                                lhsT=wk_all[:, kt, m * P : (m + 1) * P],
                                rhs=ctxT[:, kt, b, : TT + 1],
                                start=(kt == 0),
                                stop=(kt == 7),
                            )
                        nc.scalar.mul(kT[b][m][:, 0:TT], pst[:, :TT], SCALE)

                # text values: V[0:TT, :]
                for b in range(BPC):
                    psv = ps_ss.tile([P, CH], F32, tag="pss")
                    for kt in range(8):
                        nc.tensor.matmul(
                            psv[:TT, :],
                            lhsT=ctxT[:, kt, b, :TT],
                            rhs=wv_all[:, kt, :],
                            start=(kt == 0),
                            stop=(kt == 7),
                        )
                    nc.scalar.copy(V[b][0:TT, :], psv[:TT, :])

                pre_p = emit_p(0, 0)

                # image keys: kT[:, IMS:TX]
                for b in range(BPC):
                    for m in range(4):
                        psi = ps_big.tile([P, CH], F32, tag="big")
                        for kt in range(8):
                            nc.tensor.matmul(
                                psi[:, :TI],
                                lhsT=wkip_all[:, kt, m * P : (m + 1) * P],
                                rhs=ctxT[:, kt, b, TT:T],
                                start=(kt == 0),
                                stop=(kt == 7),
                            )
                        nc.scalar.mul(kT[b][m][:, IMS:TX], psi[:, :TI], SCALE)

                # image values for ALL batches in one matmul chain:
                # lhsT [128, b*16=64], out [64, 512]. Engines cannot address
                # V's partition offset 77, so bounce through SBUF + DMA.
                psw = ps_big.tile([P, CH], F32, tag="big")
                for kt in range(8):
                    nc.tensor.matmul(
                        psw[: BPC * TI, :],
                        lhsT=ctxI[:, kt, :],
                        rhs=wvip_all[:, kt, :],
                        start=(kt == 0),
                        stop=(kt == 7),
                    )
                vtmp = p0.tile([BPC * TI, INNER], F16, tag="vtmp", name="vtmp")
                nc.scalar.copy(vtmp[:], psw[: BPC * TI, :])
                for b in range(BPC):
                    nc.sync.dma_start(
                        V[b][IMS:TX, :], vtmp[b * TI : (b + 1) * TI, :]
                    )

            # ---- main loop ----
            # Step i emits: A(i) (interleaved with P(i+1)), B(i)+recips+bcast,
            # then CDF(i-1) (normalize muls, D, F) whose rb broadcast landed
            # during this step's A/P work.

            def emit_a_heads(b, c, qT, heads):
                esbs = []
                for h in heads:
                    mt, mo = h // 2, 64 * (h % 2)
                    pss = ps_ss.tile([P, CH], F32, tag="pss")
                    nc.tensor.matmul(
                        pss[:TX, :],
                        lhsT=kT[b][mt][mo : mo + 64, :],
                        rhs=qT[mo : mo + 64, mt, :],
                        start=True,
                        stop=True,
                        tile_position=(mo, 0),
                    )
                    esb = ep.tile([TX, CH], F16, tag="esb")
                    nc.scalar.activation(esb[:], pss[:TX, :], EXP)
                    esbs.append(esb)
                return esbs

            def emit_p_part(state, ms):
                b, c, qT, xT = state
                for m in ms:
                    psq = ps_big.tile([P, CH], F32, tag="big", name=f"psq{b}_{c}_{m}")
                    for kt in range(4):
                        nc.tensor.matmul(
                            psq[:],
                            lhsT=wq_all[:, kt, m * P : (m + 1) * P],
                            rhs=xT[:, kt, :],
                            start=(kt == 0),
                            stop=(kt == 3),
                        )
                    nc.scalar.copy(qT[:, m, :], psq[:])

            def emit_b_pair(esbs, rinv, hp):
                # B: key-group sums on PE (two heads per psum bank: txt/img
                # sum rows at 0:2 for even heads, 64:66 for odd -- both legal
                # output bases), one DVE reciprocal per pair.
                psr = ps_r.tile([66, CH], F32, tag="psr")
                for sub in (0, 64):
                    nc.tensor.matmul(
                        psr[sub : sub + 2, :],
                        lhsT=ind_t[:],
                        rhs=esbs[2 * hp + sub // 64][:],
                        start=True,
                        stop=True,
                    )
                with nc.allow_low_precision(
                    reason="fp16 reciprocal feeds fp16 normalize multiply"
                ):
                    nc.vector.reciprocal(rinv[:, hp, :], psr[:])

            def emit_cd_head(state, aT, h):
                b, c, esbs, rinv = state
                # C: matmul re-broadcasts reciprocals across key partitions
                # (zeroing pad rows via ind2's zero pad cols), DVE multiplies
                base = 64 * (h % 2)
                psb = ps_b.tile([TX, CH], F32, tag="psb")
                nc.tensor.matmul(
                    psb[:],
                    lhsT=ind2_t[base : base + 2, :],
                    rhs=rinv[base : base + 2, h // 2, :],
                    start=True,
                    stop=True,
                )
                nc.vector.tensor_mul(out=esbs[h][:], in0=esbs[h][:], in1=psb[:])
                mt, mo = h // 2, 64 * (h % 2)
                pso = ps_ss.tile([P, CH], F32, tag="pss")
                nc.tensor.matmul(
                    pso[:D, :],
                    lhsT=V[b][:, h * D : (h + 1) * D],
                    rhs=esbs[h][:],
                    start=True,
                    stop=True,
                )
                nc.scalar.copy(aT[mo : mo + D, mt, :], pso[:D, :])

            def emit_f(state, aT):
                b, c, esbs, rinv = state

                # final projection for this chunk
                for m in range(4):
                    psf = ps_big.tile([P, CH], F32, tag="big")
                    for kt in range(4):
                        nc.tensor.matmul(
                            psf[:],
                            lhsT=aT[:, kt, m * P : (m + 1) * P],
                            rhs=wout_all[:, kt, :],
                            start=(kt == 0),
                            stop=(kt == 3),
                        )
                    osb = osp.tile([P, DQ], F16, tag="osb")
                    nc.vector.tensor_add(out=osb[:], in0=psf[:], in1=bb_t[:])
                    nc.sync.dma_start(
                        out_d[b, c * CH + m * P : c * CH + (m + 1) * P, :],
                        osb[:],
                    )

            coords = [(b, c) for b in range(BPC) for c in range(NCH)]
            qstates = {coords[0]: pre_p}
            cdf_pend = None
            for i, (b, c) in enumerate(coords):
                bq, cq, qT = qstates.pop((b, c))
                # prefetch x two chunks ahead
                if i + 2 < len(coords):
                    fetch_x(*coords[i + 2])
                # next chunk's qT tile (projection interleaved with A heads)
                nstate = None
                if i + 1 < len(coords):
                    nb, ncc = coords[i + 1]
                    nqT = wp.tile([P, 4, CH], F16, tag="qT", name=f"qT{nb}_{ncc}")
                    nstate = (nb, ncc, nqT, xtiles.pop((nb, ncc)))
                esbs = emit_a_heads(b, c, qT, range(4))
                if nstate is not None:
                    emit_p_part(nstate, (0, 1))
                esbs += emit_a_heads(b, c, qT, range(4, 8))
                if nstate is not None:
                    emit_p_part(nstate, (2, 3))
                    qstates[coords[i + 1]] = (nstate[0], nstate[1], nstate[2])
                rinvx = rp.tile([66, H // 2, CH], F16, tag="rinv", name=f"ri{b}_{c}")
                aTp = (
                    wp.tile([P, 4, CH], F16, tag="aT", name=f"aT{b}_{c}")
                    if cdf_pend
                    else None
                )
                for hp in range(H // 2):
                    emit_b_pair(esbs, rinvx, hp)
                    if cdf_pend is not None:
                        emit_cd_head(cdf_pend, aTp, 2 * hp)
                        emit_cd_head(cdf_pend, aTp, 2 * hp + 1)
                if cdf_pend is not None:
                    emit_f(cdf_pend, aTp)
                cdf_pend = (b, c, esbs, rinvx)
            aTl = wp.tile([P, 4, CH], F16, tag="aT", name="aTlast")
            for h in range(H):
                emit_cd_head(cdf_pend, aTl, h)
            emit_f(cdf_pend, aTl)
            wstack.close()

    nc.compile()
    return nc


def _get_nc(cfg=None):
    global _CACHED
    if _CACHED is None:
        _CACHED = _build(cfg)
    return _CACHED


def _aux_inputs(b_out):
    ind = np.zeros((TX, 2), dtype=np.float16)
    ind[0:TT, 0] = 1.0
    ind[IMS:TX, 1] = 1.0
    ind2 = np.zeros((2, TX), dtype=np.float16)
    ind2[0, 0:TT] = 1.0
    ind2[1, IMS:TX] = 1.0
    bb = np.broadcast_to(np.asarray(b_out, np.float32), (P, DQ)).copy()
    return ind, ind2, bb


def run(inputs, trace=False):
    x = np.asarray(inputs["x"], dtype=np.float32)
    ctx = np.asarray(inputs["context"], dtype=np.float32)
    xT = np.ascontiguousarray(x.transpose(0, 2, 1)).astype(np.float16)
    ctxT = np.ascontiguousarray(ctx.transpose(0, 2, 1)).astype(np.float16)
    ws = {
        k: np.ascontiguousarray(np.asarray(inputs[k], dtype=np.float16))
        for k in ("W_q", "W_k", "W_v", "W_k_ip", "W_v_ip", "W_out")
    }
    ind, ind2, bb = _aux_inputs(inputs["b_out"])

    in_maps = []
    for c in range(NCORES):
        m = {
            "xT": xT[c * BPC : (c + 1) * BPC],
            "ctxT": ctxT[c * BPC : (c + 1) * BPC],
            "ctxI": np.ascontiguousarray(
                ctxT[c * BPC : (c + 1) * BPC, :, TT:].transpose(1, 0, 2).reshape(DC, BPC * TI)
            ),
            "ind": ind,
            "ind2": ind2,
            "b_bcast": bb,
        }
        m.update(ws)
        in_maps.append(m)

    nc = _get_nc()
    res = run_bass_kernel_spmd(nc, in_maps, list(range(NCORES)), trace=trace)
    out = np.concatenate([res.results[c]["out"] for c in range(NCORES)], axis=0)
    return out.astype(np.float32, copy=False), res


def kernel(**inputs):
    out, _ = run(inputs)
    return out


# revision 16
# speedup vs baseline: 1.0006x; 1.0006x over previous
"""Cross-attention kernel for Trainium2, SPMD over 8 NeuronCores.

Problem (hardcoded): B=32, N=2560 queries, Dq=512, Dc=1024, 8 heads x 64 dim,
context = 77 text + 16 image tokens, two attentions (text keys via W_k/W_v,
image keys via W_k_ip/W_v_ip) summed, then W_out projection + bias.

Sharding: data-parallel over batch, 4 batches per core, no collectives.
x and context are transposed host-side during sharding so every DMA lands in
the feature-on-partitions layout the matmuls need.

All matmul operands are fp16 (1 cycle/row on PE at ANY moving size, vs
fp32r's 4x penalty under 256; fp32 PSUM accumulate keeps precision ~1e-3).

Per-core kernel, per 512-query chunk (software-pipelined over chunks):
  P: q^T = W_q^T @ x^T                            (PE; Act evacuates)
  A: s^T[93 keys, 512 q] = k^T_h^T @ q^T_h; E = exp(s^T) -> fp16 (ScalarE)
  B: r[2, 512] = ind^T @ E  (txt/img key sums, PE); r_inv = 1/r (VectorE,
     all 8 heads into one [2,8,512] tile); r_inv is then replicated across
     key partitions by TWO partition_broadcast DMAs (txt rows 0:77 get
     r_inv[0], img rows 77:93 get r_inv[1]) -- this replaces the old
     broadcast matmul, saving 8x512 PE rows per chunk.
  C: E *= rb (VectorE, fp16)
  D: O^T = V^T @ E  (single matmul over all 93 keys sums the text and
     image attention outputs); Pool engine evacuates to attn^T fp16
  F: out = attn^T^T @ W_out; Pool adds bias, fp16 -> DMA out

Emission interleaves A/P and B/D on the PE stream so the in-order engine
never stalls on Act/DVE evacuation pacing.

PSUM budget (8 banks): big(q/final)=2, scores/attn=4, sums=2.
"""

import sys

if "/opt/trn_rl_repo" not in sys.path:
    sys.path.insert(0, "/opt/trn_rl_repo")

import numpy as np

from concourse import bacc
import concourse.mybir as mybir
from concourse.tile import TileContext
from concourse.bass_utils import run_bass_kernel_spmd

F32 = mybir.dt.float32
F16 = mybir.dt.float16
EXP = mybir.ActivationFunctionType.Exp

P = 128
NCORES = 8
B = 32
BPC = B // NCORES  # batches per core
N = 2560
DQ = 512
DC = 1024
H = 8
D = 64
INNER = H * D  # 512
TT = 77  # text tokens
TI = 16  # image tokens
T = TT + TI  # 93
# padded key layout: text keys at rows 0:77, zero pad 77:96, img at 96:112
# (engine partition windows: start 0 spans freely; start 96 spans <=32)
IMS = 96  # img key start row
TX = IMS + TI  # 112 total key rows
CH = 512  # query chunk
NCH = N // CH  # 5
SCALE = D ** (-0.5)

_CACHED = None


def _build(cfg=None):
    cfg = cfg or {}
    import contextlib
    nc = bacc.Bacc("TRN2", target_bir_lowering=False, debug=False, num_devices=NCORES)

    xt_d = nc.dram_tensor("xT", [BPC, DQ, N], F16, kind="ExternalInput").ap()
    ctxt_d = nc.dram_tensor("ctxT", [BPC, DC, T], F16, kind="ExternalInput").ap()
    wq_d = nc.dram_tensor("W_q", [DQ, INNER], F16, kind="ExternalInput").ap()
    wk_d = nc.dram_tensor("W_k", [DC, INNER], F16, kind="ExternalInput").ap()
    wv_d = nc.dram_tensor("W_v", [DC, INNER], F16, kind="ExternalInput").ap()
    wkip_d = nc.dram_tensor("W_k_ip", [DC, INNER], F16, kind="ExternalInput").ap()
    wvip_d = nc.dram_tensor("W_v_ip", [DC, INNER], F16, kind="ExternalInput").ap()
    wout_d = nc.dram_tensor("W_out", [INNER, DQ], F16, kind="ExternalInput").ap()
    ind_d = nc.dram_tensor("ind", [TX, 2], F16, kind="ExternalInput").ap()
    ind2_d = nc.dram_tensor("ind2", [2, TX], F16, kind="ExternalInput").ap()
    ctxi_d = nc.dram_tensor("ctxI", [DC, BPC * TI], F16, kind="ExternalInput").ap()
    bb_d = nc.dram_tensor("b_bcast", [P, DQ], F32, kind="ExternalInput").ap()
    out_d = nc.dram_tensor("out", [BPC, N, DQ], F16, kind="ExternalOutput").ap()

    with TileContext(nc) as tc:
        with (
            tc.tile_pool(name="persist", bufs=1) as pp,
            tc.tile_pool(name="ps_big", bufs=2, space="PSUM") as ps_big,
            tc.tile_pool(name="ps_ss", bufs=3, space="PSUM") as ps_ss,
            tc.tile_pool(name="ps_b", bufs=2, space="PSUM") as ps_b,
            tc.tile_pool(name="ps_r", bufs=1, space="PSUM") as ps_r,
        ):
            ind_t = pp.tile([TX, 2], F16, tag="ind")
            ind2_t = pp.tile([66, TX], F16, tag="ind2")
            bb_t = pp.tile([P, DQ], F32, tag="bb")

            wq_all = pp.tile([P, 4, INNER], F16, tag="wq_all")
            wout_all = pp.tile([P, 4, DQ], F16, tag="wout_all")

            # K^T[b][m] : [128 inner-dims, 93 keys] (text keys 0:77 from W_k,
            # image keys 77:93 from W_k_ip), pre-scaled by 1/sqrt(d).
            # V[b] : [93 keys, 512 inner] (text rows via W_v, image via W_v_ip)
            kT = [
                [
                    pp.tile([P, TX], F16, tag=f"kT{b}_{m}", name=f"kT{b}_{m}")
                    for m in range(4)
                ]
                for b in range(BPC)
            ]
            V = [pp.tile([TX, INNER], F16, tag=f"v{b}", name=f"v{b}") for b in range(BPC)]

            # ---- pools for the main loop (opened early so chunk (0,0)
            # projection work can interleave with phase 0) ----
            wstack = contextlib.ExitStack()
            wp = wstack.enter_context(tc.tile_pool(name="work", bufs=2))
            xsp = wstack.enter_context(tc.tile_pool(name="xsp", bufs=4))
            osp = wstack.enter_context(tc.tile_pool(name="osp", bufs=6))
            ep = wstack.enter_context(tc.tile_pool(name="ework", bufs=18))
            rp = wstack.enter_context(tc.tile_pool(name="rwork", bufs=2))

            xtiles = {}

            def fetch_x(b, c):
                # x^T for a chunk straight from DRAM: [128, kt, 512] fp16
                xT = xsp.tile([P, 4, CH], F16, tag="xT", name=f"xT{b}_{c}")
                nc.sync.dma_start(
                    xT[:],
                    xt_d[b].rearrange("(k p) t -> p k t", p=P)[
                        :, :, c * CH : (c + 1) * CH
                    ],
                )
                xtiles[(b, c)] = xT

            def emit_p(b, c):
                xT = xtiles.pop((b, c))
                # q^T chunk: [128, m, 512] fp16
                qT = wp.tile([P, 4, CH], F16, tag="qT", name=f"qT{b}_{c}")
                for m in range(4):
                    psq = ps_big.tile([P, CH], F32, tag="big", name=f"psq{b}_{c}_{m}")
                    for kt in range(4):
                        nc.tensor.matmul(
                            psq[:],
                            lhsT=wq_all[:, kt, m * P : (m + 1) * P],
                            rhs=xT[:, kt, :],
                            start=(kt == 0),
                            stop=(kt == 3),
                        )
                    nc.scalar.copy(qT[:, m, :], psq[:])
                return (b, c, qT)

            # ---- phase 0: context projections ----
            with tc.tile_pool(name="ph0", bufs=1) as p0:
                wk_all = p0.tile([P, 8, INNER], F16, tag="wk", name="wk_all")
                nc.sync.dma_start(wk_all[:], wk_d.rearrange("(k p) n -> p k n", p=P))
                # all 4 batches' context: [128, kt, b, 93]; plus a host-
                # prebatched image-token view [128, kt, b*16+i] whose single
                # contiguous free dim feeds the batched V_ip projection.
                ctxT = p0.tile([P, 8, BPC, T], F16, tag="ctxT", name="ctxT")
                for b in range(BPC):
                    nc.sync.dma_start(
                        ctxT[:, :, b, :],
                        ctxt_d[b].rearrange("(k p) t -> p k t", p=P),
                    )
                ctxI = p0.tile([P, 8, BPC * TI], F16, tag="ctxI", name="ctxI")
                nc.sync.dma_start(
                    ctxI[:], ctxi_d.rearrange("(k p) n -> p k n", p=P)
                )
                wv_all = p0.tile([P, 8, INNER], F16, tag="wv", name="wv_all")
                nc.sync.dma_start(wv_all[:], wv_d.rearrange("(k p) n -> p k n", p=P))
                nc.sync.dma_start(wq_all[:], wq_d.rearrange("(k p) n -> p k n", p=P))
                fetch_x(0, 0)
                wkip_all = p0.tile([P, 8, INNER], F16, tag="wkip", name="wkip_all")
                nc.sync.dma_start(
                    wkip_all[:], wkip_d.rearrange("(k p) n -> p k n", p=P)
                )
                wvip_all = p0.tile([P, 8, INNER], F16, tag="wvip", name="wvip_all")
                nc.sync.dma_start(
                    wvip_all[:], wvip_d.rearrange("(k p) n -> p k n", p=P)
                )
                fetch_x(0, 1)
                nc.sync.dma_start(
                    wout_all[:], wout_d.rearrange("(k p) n -> p k n", p=P)
                )
                nc.sync.dma_start(ind_t[:], ind_d)
                nc.sync.dma_start(ind2_t[0:2, :], ind2_d)
                nc.sync.dma_start(ind2_t[64:66, :], ind2_d)
                nc.sync.dma_start(bb_t[:], bb_d)

                # zero kT pad cols and V img+pad rows (img values land in
                # V[0:16] afterwards; the DMA-write ordering keeps them)
                for b in range(BPC):
                    for m in range(4):
                        nc.vector.memset(kT[b][m][:, TT:IMS], 0.0)
                    nc.gpsimd.memset(V[b][64:IMS, :], 0.0)

                # text keys: kT[:, 0:TT] (project 78 keys, junk col 77 unused)
                for b in range(BPC):
                    for m in range(4):
                        pst = ps_ss.tile([P, CH], F32, tag="pss")
                        for kt in range(8):
                            nc.tensor.matmul(
                                pst[:, : TT + 1],
                                lhsT=wk_all[:, kt, m * P : (m + 1) * P],
                                rhs=ctxT[:, kt, b, : TT + 1],
                                start=(kt == 0),
                                stop=(kt == 7),
                            )
                        nc.scalar.mul(kT[b][m][:, 0:TT], pst[:, :TT], SCALE)

                # text values: V[0:TT, :]
                for b in range(BPC):
                    psv = ps_ss.tile([P, CH], F32, tag="pss")
                    for kt in range(8):
                        nc.tensor.matmul(
                            psv[:TT, :],
                            lhsT=ctxT[:, kt, b, :TT],
                            rhs=wv_all[:, kt, :],
                            start=(kt == 0),
                            stop=(kt == 7),
                        )
                    nc.scalar.copy(V[b][0:TT, :], psv[:TT, :])

                pre_p = emit_p(0, 0)

                # image keys: kT[:, IMS:TX]
                for b in range(BPC):
                    for m in range(4):
                        psi = ps_big.tile([P, CH], F32, tag="big")
                        for kt in range(8):
                            nc.tensor.matmul(
                                psi[:, :TI],
                                lhsT=wkip_all[:, kt, m * P : (m + 1) * P],
                                rhs=ctxT[:, kt, b, TT:T],
                                start=(kt == 0),
                                stop=(kt == 7),
                            )
                        nc.scalar.mul(kT[b][m][:, IMS:TX], psi[:, :TI], SCALE)

                # image values for ALL batches in one matmul chain:
                # lhsT [128, b*16=64], out [64, 512]. Engines cannot address
                # V's partition offset 77, so bounce through SBUF + DMA.
                psw = ps_big.tile([P, CH], F32, tag="big")
                for kt in range(8):
                    nc.tensor.matmul(
                        psw[: BPC * TI, :],
                        lhsT=ctxI[:, kt, :],
                        rhs=wvip_all[:, kt, :],
                        start=(kt == 0),
                        stop=(kt == 7),
                    )
                vtmp = p0.tile([BPC * TI, INNER], F16, tag="vtmp", name="vtmp")
                nc.scalar.copy(vtmp[:], psw[: BPC * TI, :])
                for b in range(BPC):
                    nc.sync.dma_start(
                        V[b][IMS:TX, :], vtmp[b * TI : (b + 1) * TI, :]
                    )

            # ---- main loop ----
            # Step i emits: A(i) (interleaved with P(i+1)), B(i)+recips+bcast,
            # then CDF(i-1) (normalize muls, D, F) whose rb broadcast landed
            # during this step's A/P work.

            def emit_a_heads(b, c, qT, heads):
                esbs = []
                for h in heads:
                    mt, mo = h // 2, 64 * (h % 2)
                    pss = ps_ss.tile([P, CH], F32, tag="pss")
                    nc.tensor.matmul(
                        pss[:TX, :],
                        lhsT=kT[b][mt][mo : mo + 64, :],
                        rhs=qT[mo : mo + 64, mt, :],
                        start=True,
                        stop=True,
                        tile_position=(mo, 0),
                    )
                    esb = ep.tile([TX, CH], F16, tag="esb")
                    nc.scalar.activation(esb[:], pss[:TX, :], EXP)
                    esbs.append(esb)
                return esbs

            def emit_p_part(state, ms):
                b, c, qT, xT = state
                for m in ms:
                    psq = ps_big.tile([P, CH], F32, tag="big", name=f"psq{b}_{c}_{m}")
                    for kt in range(4):
                        nc.tensor.matmul(
                            psq[:],
                            lhsT=wq_all[:, kt, m * P : (m + 1) * P],
                            rhs=xT[:, kt, :],
                            start=(kt == 0),
                            stop=(kt == 3),
                        )
                    nc.scalar.copy(qT[:, m, :], psq[:])

            def emit_b_pair(esbs, rinv, hp):
                # B: key-group sums on PE (two heads per psum bank: txt/img
                # sum rows at 0:2 for even heads, 64:66 for odd -- both legal
                # output bases), one DVE reciprocal per pair.
                psr = ps_r.tile([66, CH], F32, tag="psr")
                for sub in (0, 64):
                    nc.tensor.matmul(
                        psr[sub : sub + 2, :],
                        lhsT=ind_t[:],
                        rhs=esbs[2 * hp + sub // 64][:],
                        start=True,
                        stop=True,
                    )
                with nc.allow_low_precision(
                    reason="fp16 reciprocal feeds fp16 normalize multiply"
                ):
                    nc.vector.reciprocal(rinv[:, hp, :], psr[:])

            def emit_cd_head(state, aT, h):
                b, c, esbs, rinv = state
                # C: matmul re-broadcasts reciprocals across key partitions
                # (zeroing pad rows via ind2's zero pad cols), DVE multiplies
                base = 64 * (h % 2)
                psb = ps_b.tile([TX, CH], F32, tag="psb")
                nc.tensor.matmul(
                    psb[:],
                    lhsT=ind2_t[base : base + 2, :],
                    rhs=rinv[base : base + 2, h // 2, :],
                    start=True,
                    stop=True,
                )
                nc.vector.tensor_mul(out=esbs[h][:], in0=esbs[h][:], in1=psb[:])
                mt, mo = h // 2, 64 * (h % 2)
                pso = ps_ss.tile([P, CH], F32, tag="pss")
                nc.tensor.matmul(
                    pso[:D, :],
                    lhsT=V[b][:, h * D : (h + 1) * D],
                    rhs=esbs[h][:],
                    start=True,
                    stop=True,
                )
                nc.scalar.copy(aT[mo : mo + D, mt, :], pso[:D, :])

            def emit_f(state, aT):
                b, c, esbs, rinv = state

                # final projection for this chunk
                for m in range(4):
                    psf = ps_big.tile([P, CH], F32, tag="big")
                    for kt in range(4):
                        nc.tensor.matmul(
                            psf[:],
                            lhsT=aT[:, kt, m * P : (m + 1) * P],
                            rhs=wout_all[:, kt, :],
                            start=(kt == 0),
                            stop=(kt == 3),
                        )
                    osb = osp.tile([P, DQ], F16, tag="osb")
                    nc.vector.tensor_add(out=osb[:], in0=psf[:], in1=bb_t[:])
                    nc.sync.dma_start(
                        out_d[b, c * CH + m * P : c * CH + (m + 1) * P, :],
                        osb[:],
                    )

            coords = [(b, c) for b in range(BPC) for c in range(NCH)]
            qstates = {coords[0]: pre_p}
            cdf_pend = None
            for i, (b, c) in enumerate(coords):
                bq, cq, qT = qstates.pop((b, c))
                # prefetch x two chunks ahead
                if i + 2 < len(coords):
                    fetch_x(*coords[i + 2])
                # next chunk's qT tile (projection interleaved with A heads)
                nstate = None
                if i + 1 < len(coords):
                    nb, ncc = coords[i + 1]
                    nqT = wp.tile([P, 4, CH], F16, tag="qT", name=f"qT{nb}_{ncc}")
                    nstate = (nb, ncc, nqT, xtiles.pop((nb, ncc)))
                esbs = emit_a_heads(b, c, qT, range(4))
                if nstate is not None:
                    emit_p_part(nstate, (0, 1))
                esbs += emit_a_heads(b, c, qT, range(4, 8))
                if nstate is not None:
                    emit_p_part(nstate, (2, 3))
                    qstates[coords[i + 1]] = (nstate[0], nstate[1], nstate[2])
                rinvx = rp.tile([66, H // 2, CH], F16, tag="rinv", name=f"ri{b}_{c}")
                aTp = (
                    wp.tile([P, 4, CH], F16, tag="aT", name=f"aT{b}_{c}")
                    if cdf_pend
                    else None
                )
                for hp in range(H // 2):
                    emit_b_pair(esbs, rinvx, hp)
                if cdf_pend is not None:
                    for h in range(H):
                        emit_cd_head(cdf_pend, aTp, h)
                    emit_f(cdf_pend, aTp)
                cdf_pend = (b, c, esbs, rinvx)
            aTl = wp.tile([P, 4, CH], F16, tag="aT", name="aTlast")
            for h in range(H):
                emit_cd_head(cdf_pend, aTl, h)
            emit_f(cdf_pend, aTl)
            wstack.close()

    nc.compile()
    return nc


def _get_nc(cfg=None):
    global _CACHED
    if _CACHED is None:
        _CACHED = _build(cfg)
    return _CACHED


def _aux_inputs(b_out):
    ind = np.zeros((TX, 2), dtype=np.float16)
    ind[0:TT, 0] = 1.0
    ind[IMS:TX, 1] = 1.0
    ind2 = np.zeros((2, TX), dtype=np.float16)
    ind2[0, 0:TT] = 1.0
    ind2[1, IMS:TX] = 1.0
    bb = np.broadcast_to(np.asarray(b_out, np.float32), (P, DQ)).copy()
    return ind, ind2, bb


def run(inputs, trace=False):
    x = np.asarray(inputs["x"], dtype=np.float32)
    ctx = np.asarray(inputs["context"], dtype=np.float32)
    xT = np.ascontiguousarray(x.transpose(0, 2, 1)).astype(np.float16)
    ctxT = np.ascontiguousarray(ctx.transpose(0, 2, 1)).astype(np.float16)
    ws = {
        k: np.ascontiguousarray(np.asarray(inputs[k], dtype=np.float16))
        for k in ("W_q", "W_k", "W_v", "W_k_ip", "W_v_ip", "W_out")
    }
    ind, ind2, bb = _aux_inputs(inputs["b_out"])

    in_maps = []
    for c in range(NCORES):
        m = {
            "xT": xT[c * BPC : (c + 1) * BPC],
            "ctxT": ctxT[c * BPC : (c + 1) * BPC],
            "ctxI": np.ascontiguousarray(
                ctxT[c * BPC : (c + 1) * BPC, :, TT:].transpose(1, 0, 2).reshape(DC, BPC * TI)
            ),
            "ind": ind,
            "ind2": ind2,
            "b_bcast": bb,
        }
        m.update(ws)
        in_maps.append(m)

    nc = _get_nc()
    res = run_bass_kernel_spmd(nc, in_maps, list(range(NCORES)), trace=trace)
    out = np.concatenate([res.results[c]["out"] for c in range(NCORES)], axis=0)
    return out.astype(np.float32, copy=False), res


def kernel(**inputs):
    out, _ = run(inputs)
    return out


# revision 17
# speedup vs baseline: 1.0396x; 1.0390x over previous
"""Cross-attention kernel for Trainium2, SPMD over 8 NeuronCores.

Problem (hardcoded): B=32, N=2560 queries, Dq=512, Dc=1024, 8 heads x 64 dim,
context = 77 text + 16 image tokens, two attentions (text keys via W_k/W_v,
image keys via W_k_ip/W_v_ip) summed, then W_out projection + bias.

Sharding: data-parallel over batch, 4 batches per core, no collectives.
x and context are transposed host-side during sharding so every DMA lands in
the feature-on-partitions layout the matmuls need.

All matmul operands are fp16 (1 cycle/row on PE at ANY moving size, vs
fp32r's 4x penalty under 256; fp32 PSUM accumulate keeps precision ~1e-3).

Per-core kernel, per 512-query chunk (software-pipelined over chunks):
  P: q^T = W_q^T @ x^T                            (PE; Act evacuates)
  A: s^T[93 keys, 512 q] = k^T_h^T @ q^T_h; E = exp(s^T) -> fp16 (ScalarE)
  B: r[2, 512] = ind^T @ E  (txt/img key sums, PE); r_inv = 1/r (VectorE,
     all 8 heads into one [2,8,512] tile); r_inv is then replicated across
     key partitions by TWO partition_broadcast DMAs (txt rows 0:77 get
     r_inv[0], img rows 77:93 get r_inv[1]) -- this replaces the old
     broadcast matmul, saving 8x512 PE rows per chunk.
  C: E *= rb (VectorE, fp16)
  D: O^T = V^T @ E  (single matmul over all 93 keys sums the text and
     image attention outputs); Pool engine evacuates to attn^T fp16
  F: out = attn^T^T @ W_out; Pool adds bias, fp16 -> DMA out

Emission interleaves A/P and B/D on the PE stream so the in-order engine
never stalls on Act/DVE evacuation pacing.

PSUM budget (8 banks): big(q/final)=2, scores/attn=4, sums=2.
"""

import sys

if "/opt/trn_rl_repo" not in sys.path:
    sys.path.insert(0, "/opt/trn_rl_repo")

import numpy as np

from concourse import bacc
import concourse.mybir as mybir
from concourse.tile import TileContext
from concourse.bass_utils import run_bass_kernel_spmd

F32 = mybir.dt.float32
F16 = mybir.dt.float16
EXP = mybir.ActivationFunctionType.Exp

P = 128
NCORES = 8
B = 32
BPC = B // NCORES  # batches per core
N = 2560
DQ = 512
DC = 1024
H = 8
D = 64
INNER = H * D  # 512
TT = 77  # text tokens
TI = 16  # image tokens
T = TT + TI  # 93
# padded key layout: text keys at rows 0:77, zero pad 77:96, img at 96:112
# (engine partition windows: start 0 spans freely; start 96 spans <=32)
IMS = 96  # img key start row
TX = IMS + TI  # 112 total key rows
CH = 512  # query chunk
NCH = N // CH  # 5
SCALE = D ** (-0.5)

_CACHED = None


def _build(cfg=None):
    cfg = cfg or {}
    import contextlib
    nc = bacc.Bacc("TRN2", target_bir_lowering=False, debug=False, num_devices=NCORES)

    xt_d = nc.dram_tensor("xT", [BPC, DQ, N], F16, kind="ExternalInput").ap()
    ctxt_d = nc.dram_tensor("ctxT", [BPC, DC, T], F16, kind="ExternalInput").ap()
    wq_d = nc.dram_tensor("W_q", [DQ, INNER], F16, kind="ExternalInput").ap()
    wk_d = nc.dram_tensor("W_k", [DC, INNER], F16, kind="ExternalInput").ap()
    wv_d = nc.dram_tensor("W_v", [DC, INNER], F16, kind="ExternalInput").ap()
    wkip_d = nc.dram_tensor("W_k_ip", [DC, INNER], F16, kind="ExternalInput").ap()
    wvip_d = nc.dram_tensor("W_v_ip", [DC, INNER], F16, kind="ExternalInput").ap()
    wout_d = nc.dram_tensor("W_out", [INNER, DQ], F16, kind="ExternalInput").ap()
    ind_d = nc.dram_tensor("ind", [TX, 2], F16, kind="ExternalInput").ap()
    ind2_d = nc.dram_tensor("ind2", [2, TX], F16, kind="ExternalInput").ap()
    ctxi_d = nc.dram_tensor("ctxI", [DC, BPC * TI], F16, kind="ExternalInput").ap()
    bb_d = nc.dram_tensor("b_bcast", [P, DQ], F32, kind="ExternalInput").ap()
    out_d = nc.dram_tensor("out", [BPC, N, DQ], F16, kind="ExternalOutput").ap()

    with TileContext(nc) as tc:
        with (
            tc.tile_pool(name="persist", bufs=1) as pp,
            tc.tile_pool(name="ps_big", bufs=2, space="PSUM") as ps_big,
            tc.tile_pool(name="ps_ss", bufs=3, space="PSUM") as ps_ss,
            tc.tile_pool(name="ps_b", bufs=2, space="PSUM") as ps_b,
            tc.tile_pool(name="ps_r", bufs=1, space="PSUM") as ps_r,
        ):
            ind_t = pp.tile([TX, 2], F16, tag="ind")
            ind2_t = pp.tile([66, TX], F16, tag="ind2")
            bb_t = pp.tile([P, DQ], F32, tag="bb")

            wq_all = pp.tile([P, 4, INNER], F16, tag="wq_all")
            wout_all = pp.tile([P, 4, DQ], F16, tag="wout_all")

            # K^T[b][m] : [128 inner-dims, 93 keys] (text keys 0:77 from W_k,
            # image keys 77:93 from W_k_ip), pre-scaled by 1/sqrt(d).
            # V[b] : [93 keys, 512 inner] (text rows via W_v, image via W_v_ip)
            kT = [
                [
                    pp.tile([P, TX], F16, tag=f"kT{b}_{m}", name=f"kT{b}_{m}")
                    for m in range(4)
                ]
                for b in range(BPC)
            ]
            V = [pp.tile([TX, INNER], F16, tag=f"v{b}", name=f"v{b}") for b in range(BPC)]

            # ---- pools for the main loop (opened early so chunk (0,0)
            # projection work can interleave with phase 0) ----
            wstack = contextlib.ExitStack()
            wp = wstack.enter_context(tc.tile_pool(name="work", bufs=2))
            xsp = wstack.enter_context(tc.tile_pool(name="xsp", bufs=4))
            osp = wstack.enter_context(tc.tile_pool(name="osp", bufs=6))
            ep = wstack.enter_context(tc.tile_pool(name="ework", bufs=18))
            rp = wstack.enter_context(tc.tile_pool(name="rwork", bufs=2))

            xtiles = {}

            def fetch_x(b, c):
                # x^T for a chunk straight from DRAM: [128, kt, 512] fp16
                xT = xsp.tile([P, 4, CH], F16, tag="xT", name=f"xT{b}_{c}")
                nc.sync.dma_start(
                    xT[:],
                    xt_d[b].rearrange("(k p) t -> p k t", p=P)[
                        :, :, c * CH : (c + 1) * CH
                    ],
                )
                xtiles[(b, c)] = xT

            def emit_p(b, c):
                xT = xtiles.pop((b, c))
                # q^T chunk: [128, m, 512] fp16
                qT = wp.tile([P, 4, CH], F16, tag="qT", name=f"qT{b}_{c}")
                for m in range(4):
                    psq = ps_big.tile([P, CH], F32, tag="big", name=f"psq{b}_{c}_{m}")
                    for kt in range(4):
                        nc.tensor.matmul(
                            psq[:],
                            lhsT=wq_all[:, kt, m * P : (m + 1) * P],
                            rhs=xT[:, kt, :],
                            start=(kt == 0),
                            stop=(kt == 3),
                        )
                    nc.scalar.copy(qT[:, m, :], psq[:])
                return (b, c, qT)

            # ---- phase 0: context projections ----
            with tc.tile_pool(name="ph0", bufs=1) as p0:
                wk_all = p0.tile([P, 8, INNER], F16, tag="wk", name="wk_all")
                nc.sync.dma_start(wk_all[:], wk_d.rearrange("(k p) n -> p k n", p=P))
                # all 4 batches' context: [128, kt, b, 93]; plus a host-
                # prebatched image-token view [128, kt, b*16+i] whose single
                # contiguous free dim feeds the batched V_ip projection.
                ctxT = p0.tile([P, 8, BPC, T], F16, tag="ctxT", name="ctxT")
                for b in range(BPC):
                    nc.sync.dma_start(
                        ctxT[:, :, b, :],
                        ctxt_d[b].rearrange("(k p) t -> p k t", p=P),
                    )
                ctxI = p0.tile([P, 8, BPC * TI], F16, tag="ctxI", name="ctxI")
                nc.sync.dma_start(
                    ctxI[:], ctxi_d.rearrange("(k p) n -> p k n", p=P)
                )
                wv_all = p0.tile([P, 8, INNER], F16, tag="wv", name="wv_all")
                nc.sync.dma_start(wv_all[:], wv_d.rearrange("(k p) n -> p k n", p=P))
                nc.sync.dma_start(wq_all[:], wq_d.rearrange("(k p) n -> p k n", p=P))
                fetch_x(0, 0)
                wkip_all = p0.tile([P, 8, INNER], F16, tag="wkip", name="wkip_all")
                nc.sync.dma_start(
                    wkip_all[:], wkip_d.rearrange("(k p) n -> p k n", p=P)
                )
                wvip_all = p0.tile([P, 8, INNER], F16, tag="wvip", name="wvip_all")
                nc.sync.dma_start(
                    wvip_all[:], wvip_d.rearrange("(k p) n -> p k n", p=P)
                )
                fetch_x(0, 1)
                nc.sync.dma_start(
                    wout_all[:], wout_d.rearrange("(k p) n -> p k n", p=P)
                )
                nc.sync.dma_start(ind_t[:], ind_d)
                nc.sync.dma_start(ind2_t[0:2, :], ind2_d)
                nc.sync.dma_start(ind2_t[64:66, :], ind2_d)
                nc.sync.dma_start(bb_t[:], bb_d)

                # zero kT pad cols and V img+pad rows (img values land in
                # V[0:16] afterwards; the DMA-write ordering keeps them)
                for b in range(BPC):
                    for m in range(4):
                        nc.vector.memset(kT[b][m][:, TT:IMS], 0.0)
                    nc.gpsimd.memset(V[b][64:IMS, :], 0.0)

                # text keys: kT[:, 0:TT] (project 78 keys, junk col 77 unused)
                for b in range(BPC):
                    for m in range(4):
                        pst = ps_ss.tile([P, CH], F32, tag="pss")
                        for kt in range(8):
                            nc.tensor.matmul(
                                pst[:, : TT + 1],
                                lhsT=wk_all[:, kt, m * P : (m + 1) * P],
                                rhs=ctxT[:, kt, b, : TT + 1],
                                start=(kt == 0),
                                stop=(kt == 7),
                            )
                        nc.scalar.mul(kT[b][m][:, 0:TT], pst[:, :TT], SCALE)

                # text values: V[0:TT, :]
                for b in range(BPC):
                    psv = ps_ss.tile([P, CH], F32, tag="pss")
                    for kt in range(8):
                        nc.tensor.matmul(
                            psv[:TT, :],
                            lhsT=ctxT[:, kt, b, :TT],
                            rhs=wv_all[:, kt, :],
                            start=(kt == 0),
                            stop=(kt == 7),
                        )
                    nc.scalar.copy(V[b][0:TT, :], psv[:TT, :])

                pre_p = emit_p(0, 0)

                # image keys: kT[:, IMS:TX]
                for b in range(BPC):
                    for m in range(4):
                        psi = ps_big.tile([P, CH], F32, tag="big")
                        for kt in range(8):
                            nc.tensor.matmul(
                                psi[:, :TI],
                                lhsT=wkip_all[:, kt, m * P : (m + 1) * P],
                                rhs=ctxT[:, kt, b, TT:T],
                                start=(kt == 0),
                                stop=(kt == 7),
                            )
                        nc.scalar.mul(kT[b][m][:, IMS:TX], psi[:, :TI], SCALE)

                # image values for ALL batches in one matmul chain:
                # lhsT [128, b*16=64], out [64, 512]. Engines cannot address
                # V's partition offset 77, so bounce through SBUF + DMA.
                psw = ps_big.tile([P, CH], F32, tag="big")
                for kt in range(8):
                    nc.tensor.matmul(
                        psw[: BPC * TI, :],
                        lhsT=ctxI[:, kt, :],
                        rhs=wvip_all[:, kt, :],
                        start=(kt == 0),
                        stop=(kt == 7),
                    )
                vtmp = p0.tile([BPC * TI, INNER], F16, tag="vtmp", name="vtmp")
                nc.scalar.copy(vtmp[:], psw[: BPC * TI, :])
                for b in range(BPC):
                    nc.sync.dma_start(
                        V[b][IMS:TX, :], vtmp[b * TI : (b + 1) * TI, :]
                    )

            # ---- main loop ----
            # Step i emits: A(i) (interleaved with P(i+1)), B(i)+recips+bcast,
            # then CDF(i-1) (normalize muls, D, F) whose rb broadcast landed
            # during this step's A/P work.

            def emit_a_heads(b, c, qT, heads):
                esbs = []
                for h in heads:
                    mt, mo = h // 2, 64 * (h % 2)
                    pss = ps_ss.tile([P, CH], F32, tag="pss")
                    nc.tensor.matmul(
                        pss[:TX, :],
                        lhsT=kT[b][mt][mo : mo + 64, :],
                        rhs=qT[mo : mo + 64, mt, :],
                        start=True,
                        stop=True,
                        tile_position=(mo, 0),
                    )
                    esb = ep.tile([TX, CH], F16, tag="esb")
                    nc.scalar.activation(esb[:], pss[:TX, :], EXP)
                    esbs.append(esb)
                return esbs

            def emit_p_part(state, ms):
                b, c, qT, xT = state
                for m in ms:
                    psq = ps_big.tile([P, CH], F32, tag="big", name=f"psq{b}_{c}_{m}")
                    for kt in range(4):
                        nc.tensor.matmul(
                            psq[:],
                            lhsT=wq_all[:, kt, m * P : (m + 1) * P],
                            rhs=xT[:, kt, :],
                            start=(kt == 0),
                            stop=(kt == 3),
                        )
                    nc.scalar.copy(qT[:, m, :], psq[:])

            def emit_b_pair(esbs, rinv, hp):
                # B: key-group sums on PE (two heads per psum bank: txt/img
                # sum rows at 0:2 for even heads, 64:66 for odd -- both legal
                # output bases), one DVE reciprocal per pair.
                psr = ps_r.tile([66, CH], F32, tag="psr")
                for sub in (0, 64):
                    nc.tensor.matmul(
                        psr[sub : sub + 2, :],
                        lhsT=ind_t[:],
                        rhs=esbs[2 * hp + sub // 64][:],
                        start=True,
                        stop=True,
                    )
                with nc.allow_low_precision(
                    reason="fp16 reciprocal feeds fp16 normalize multiply"
                ):
                    nc.vector.reciprocal(rinv[:, hp, :], psr[:])

            def emit_c_head(state, h):
                b, c, esbs, rinv = state
                # C: matmul re-broadcasts reciprocals across key partitions
                # (zeroing pad rows via ind2's zero pad cols), DVE multiplies
                base = 64 * (h % 2)
                psb = ps_b.tile([TX, CH], F32, tag="psb")
                nc.tensor.matmul(
                    psb[:],
                    lhsT=ind2_t[base : base + 2, :],
                    rhs=rinv[base : base + 2, h // 2, :],
                    start=True,
                    stop=True,
                )
                nc.vector.tensor_mul(out=esbs[h][:], in0=esbs[h][:], in1=psb[:])

            def emit_d_head(state, aT, h):
                b, c, esbs, rinv = state
                mt, mo = h // 2, 64 * (h % 2)
                pso = ps_ss.tile([P, CH], F32, tag="pss")
                nc.tensor.matmul(
                    pso[:D, :],
                    lhsT=V[b][:, h * D : (h + 1) * D],
                    rhs=esbs[h][:],
                    start=True,
                    stop=True,
                )
                nc.scalar.copy(aT[mo : mo + D, mt, :], pso[:D, :])

            def emit_f(state, aT):
                b, c, esbs, rinv = state

                # final projection for this chunk
                for m in range(4):
                    psf = ps_big.tile([P, CH], F32, tag="big")
                    for kt in range(4):
                        nc.tensor.matmul(
                            psf[:],
                            lhsT=aT[:, kt, m * P : (m + 1) * P],
                            rhs=wout_all[:, kt, :],
                            start=(kt == 0),
                            stop=(kt == 3),
                        )
                    osb = osp.tile([P, DQ], F16, tag="osb")
                    nc.vector.tensor_add(out=osb[:], in0=psf[:], in1=bb_t[:])
                    nc.sync.dma_start(
                        out_d[b, c * CH + m * P : c * CH + (m + 1) * P, :],
                        osb[:],
                    )

            coords = [(b, c) for b in range(BPC) for c in range(NCH)]
            qstates = {coords[0]: pre_p}
            cdf_pend = None
            for i, (b, c) in enumerate(coords):
                bq, cq, qT = qstates.pop((b, c))
                # prefetch x two chunks ahead
                if i + 2 < len(coords):
                    fetch_x(*coords[i + 2])
                # next chunk's qT tile (projection interleaved with A heads)
                nstate = None
                if i + 1 < len(coords):
                    nb, ncc = coords[i + 1]
                    nqT = wp.tile([P, 4, CH], F16, tag="qT", name=f"qT{nb}_{ncc}")
                    nstate = (nb, ncc, nqT, xtiles.pop((nb, ncc)))
                esbs = emit_a_heads(b, c, qT, range(4))
                if nstate is not None:
                    emit_p_part(nstate, (0, 1))
                esbs += emit_a_heads(b, c, qT, range(4, 8))
                if nstate is not None:
                    emit_p_part(nstate, (2, 3))
                    qstates[coords[i + 1]] = (nstate[0], nstate[1], nstate[2])
                rinvx = rp.tile([66, H // 2, CH], F16, tag="rinv", name=f"ri{b}_{c}")
                aTp = (
                    wp.tile([P, 4, CH], F16, tag="aT", name=f"aT{b}_{c}")
                    if cdf_pend
                    else None
                )
                for hp in range(H // 2):
                    emit_b_pair(esbs, rinvx, hp)
                if cdf_pend is not None:
                    for h in range(H):
                        emit_c_head(cdf_pend, h)
                    for h in range(H):
                        emit_d_head(cdf_pend, aTp, h)
                    emit_f(cdf_pend, aTp)
                cdf_pend = (b, c, esbs, rinvx)
            aTl = wp.tile([P, 4, CH], F16, tag="aT", name="aTlast")
            for h in range(H):
                emit_c_head(cdf_pend, h)
            for h in range(H):
                emit_d_head(cdf_pend, aTl, h)
            emit_f(cdf_pend, aTl)
            wstack.close()

    nc.compile()
    return nc


def _get_nc(cfg=None):
    global _CACHED
    if _CACHED is None:
        _CACHED = _build(cfg)
    return _CACHED


def _aux_inputs(b_out):
    ind = np.zeros((TX, 2), dtype=np.float16)
    ind[0:TT, 0] = 1.0
    ind[IMS:TX, 1] = 1.0
    ind2 = np.zeros((2, TX), dtype=np.float16)
    ind2[0, 0:TT] = 1.0
    ind2[1, IMS:TX] = 1.0
    bb = np.broadcast_to(np.asarray(b_out, np.float32), (P, DQ)).copy()
    return ind, ind2, bb


def run(inputs, trace=False):
    x = np.asarray(inputs["x"], dtype=np.float32)
    ctx = np.asarray(inputs["context"], dtype=np.float32)
    xT = np.ascontiguousarray(x.transpose(0, 2, 1)).astype(np.float16)
    ctxT = np.ascontiguousarray(ctx.transpose(0, 2, 1)).astype(np.float16)
    ws = {
        k: np.ascontiguousarray(np.asarray(inputs[k], dtype=np.float16))
        for k in ("W_q", "W_k", "W_v", "W_k_ip", "W_v_ip", "W_out")
    }
    ind, ind2, bb = _aux_inputs(inputs["b_out"])

    in_maps = []
    for c in range(NCORES):
        m = {
            "xT": xT[c * BPC : (c + 1) * BPC],
            "ctxT": ctxT[c * BPC : (c + 1) * BPC],
            "ctxI": np.ascontiguousarray(
                ctxT[c * BPC : (c + 1) * BPC, :, TT:].transpose(1, 0, 2).reshape(DC, BPC * TI)
            ),
            "ind": ind,
            "ind2": ind2,
            "b_bcast": bb,
        }
        m.update(ws)
        in_maps.append(m)

    nc = _get_nc()
    res = run_bass_kernel_spmd(nc, in_maps, list(range(NCORES)), trace=trace)
    out = np.concatenate([res.results[c]["out"] for c in range(NCORES)], axis=0)
    return out.astype(np.float32, copy=False), res


def kernel(**inputs):
    out, _ = run(inputs)
    return out
